# revision 1
# baseline (speedup 1.0000x reference)
"""EventDrivenODECell Trainium2 kernel.

Math (reference semantics):
  dt = (t_end - t_start)/5
  5 Euler steps: h += dt * (W3 tanh(W2 tanh(W1a h + [bd1 + W1b te(t)])) + bd3)
    where te(t) depends only on the scalar t -> folded on host into a
    per-step bias  b1s = bd1 + W1b @ te(t_s);  dt folded into W3/bd3.
  event: out = h + sigmoid(Wg ef + bg) * (We2 relu(We1h h + We1e ef + be1) + be2)

Device layout: feature-major activations [feat, batch]; batch sharded 8 ways
(8192 rows/core) and processed in 8 column-chunks of 1024 rows. PSUM tiles
are [128, 2, 512] (2 banks): each N=512 matmul targets one bank, while ACT /
DVE drain the pair in a single 1024-wide op (the pair shares the m-half, so
the per-partition bias stays valid). Matmul inputs are float16 (full PE rate,
FWL weight loads, ~10-bit mantissa, exact fp32 PSUM accumulation); h is
accumulated in f16 by the DVE update op; biases and PSUM stay fp32.
tanh/sigmoid on ACT (bias fused); relu and all elementwise adds on DVE.
"""

import os
import sys

sys.path.insert(0, "/opt/trn_rl_repo")

import numpy as np

import concourse.bacc as bacc
import concourse.mybir as mybir
import concourse.tile as tile
from concourse.bass_utils import run_bass_kernel_spmd

B = 65536
HID = 256
EVT = 64
TEMB = 32
NUM_STEPS = 5
N_CORES = 8
R = B // N_CORES          # rows per core
S = 512                   # matmul moving-dim / PSUM bank quantum
CHUNK = 1024              # rows per processing chunk (= 2 PSUM banks)
NS = CHUNK // S           # N-splits per chunk
N_CHUNKS = R // CHUNK     # 8
GROUP = 4                 # chunks per layer-sweep group

MODE = os.environ.get("KMODE", "f16")   # "f16" | "f32r" | "f32"

f32 = mybir.dt.float32
f32r = mybir.dt.float32r
f16 = mybir.dt.float16

_CACHE = {}

# bias-pack column indices
COL_B1S = 0          # 0..4: per-step layer-1 bias
COL_B2 = 5
COL_B3 = 6
COL_BE1 = 7
COL_BE2 = 8
COL_BG = 9
N_BIAS_COLS = 10


def _build(mode):
    wdt = {"f32r": f32r, "f32": f32, "f16": f16}[mode]
    nc = bacc.Bacc("TRN2", target_bir_lowering=False, debug=False,
                   num_devices=N_CORES)

    hT_d = nc.dram_tensor("hT", [HID, R], wdt, kind="ExternalInput")
    efT_d = nc.dram_tensor("efT", [EVT, R], wdt, kind="ExternalInput")
    w1_d = nc.dram_tensor("w1", [HID, HID], wdt, kind="ExternalInput")
    w2_d = nc.dram_tensor("w2", [HID, HID], wdt, kind="ExternalInput")
    w3_d = nc.dram_tensor("w3", [HID, HID], wdt, kind="ExternalInput")
    we1h_d = nc.dram_tensor("we1h", [HID, HID], wdt, kind="ExternalInput")
    we1e_d = nc.dram_tensor("we1e", [EVT, HID], wdt, kind="ExternalInput")
    we2_d = nc.dram_tensor("we2", [HID, HID], wdt, kind="ExternalInput")
    wg_d = nc.dram_tensor("wg", [EVT, HID], wdt, kind="ExternalInput")
    biasp_d = nc.dram_tensor("biasp", [HID, N_BIAS_COLS], f32,
                             kind="ExternalInput")
    outT_d = nc.dram_tensor("outT", [HID, R], f32, kind="ExternalOutput")

    Tanh = mybir.ActivationFunctionType.Tanh
    Sigmoid = mybir.ActivationFunctionType.Sigmoid
    Relu = mybir.ActivationFunctionType.Relu
    add = mybir.AluOpType.add
    mult = mybir.AluOpType.mult
    vmax = mybir.AluOpType.max

    with tile.TileContext(nc) as tc:
        with (
            tc.tile_pool(name="consts", bufs=1) as consts,
            tc.tile_pool(name="h", bufs=1) as h_pool,
            tc.tile_pool(name="z1", bufs=18) as z1_pool,
            tc.tile_pool(name="z2", bufs=20) as z2_pool,
            tc.tile_pool(name="efc", bufs=8) as ef_pool,
            tc.tile_pool(name="stage", bufs=6) as stage_pool,
            tc.tile_pool(name="psum", bufs=4, space="PSUM") as psum_pool,
        ):
            # ---- constants / h tiles; DMA issue order matters (single
            # HWDGE queue serializes) so interleave with first-use order ----
            def load_w(d, name, kparts, kdim=128):
                ts = []
                for k in range(kparts):
                    t = consts.tile([kdim, HID], wdt, tag=f"{name}{k}",
                                    name=f"{name}{k}")
                    nc.sync.dma_start(t[:], d.ap()[k * kdim:(k + 1) * kdim, :])
                    ts.append(t)
                return ts

            h = [[h_pool.tile([128, NS, S], wdt, tag=f"h{c}_{m}",
                              name=f"h{c}_{m}")
                  for m in range(2)] for c in range(N_CHUNKS)]

            def load_h(c):
                for m in range(2):
                    nc.sync.dma_start(
                        h[c][m][:],
                        hT_d.ap()[m * 128:(m + 1) * 128,
                                  c * CHUNK:(c + 1) * CHUNK])

            w1 = load_w(w1_d, "w1", 2)
            load_h(0)
            load_h(1)
            biasp = []
            for m in range(2):
                t = consts.tile([128, N_BIAS_COLS], f32, tag=f"biasp{m}",
                                name=f"biasp{m}")
                nc.sync.dma_start(t[:], biasp_d.ap()[m * 128:(m + 1) * 128, :])
                biasp.append(t)
            w2 = load_w(w2_d, "w2", 2)
            load_h(2)
            w3 = load_w(w3_d, "w3", 2)
            load_h(3)
            we1h = load_w(we1h_d, "we1h", 2)
            we2 = load_w(we2_d, "we2", 2)
            # EVT-dim weights live in both partition halves so the two
            # m-half K=64 matmuls can run on distinct PE row groups.
            def load_evt_w(d, name):
                t = consts.tile([128, HID], wdt, tag=name, name=name)
                nc.sync.dma_start(t[0:EVT, :], d.ap())
                nc.sync.dma_start(t[EVT:128, :], d.ap())
                return t

            we1e = load_evt_w(we1e_d, "we1e")   # [128, 256], duplicated rows
            wg = load_evt_w(wg_d, "wg")
            for c in range(4, N_CHUNKS):
                load_h(c)

            def bcol(m, col):
                return biasp[m][:, col:col + 1]

            # ---- PE warmup: dependency-free junk matmuls keep the PE busy
            # while the first h/w DMAs land, and ramp HAM to full clock ----
            warm = consts.tile([128, S], wdt, tag="warm", name="warm")
            nc.vector.memset(warm[:], 0.0)
            wps = psum_pool.tile([128, S], f32, tag="ps", name="wps")
            for _ in range(16):
                nc.tensor.matmul(wps[:], warm[:, :128], warm[:],
                                 start=True, stop=True)
            # prefetch the ACT function tables while ACT is idle: the tanh
            # load otherwise lands on the first L1 drain (~14us) and the
            # sigmoid load on the first event's critical path
            wz = stage_pool.tile([128, S], f32, tag="st", name="wz")
            nc.scalar.activation(wz[:], warm[:], Tanh)
            nc.scalar.activation(wz[:], warm[:], Sigmoid)

            def mm_chunk(ps, win, x, m, kparts=2, extra=None):
                """ps [128,NS,S] (PSUM) += win[k][:, m-blk].T @ x[k] per
                N-split; optional extra=(w_evt, ef_tile) accumulated last."""
                n_acc = kparts + (1 if extra is not None else 0)
                for k in range(kparts):
                    wblk = win[k][:, m * 128:(m + 1) * 128]
                    for j in range(NS):
                        nc.tensor.matmul(ps[:, j], wblk, x[k][:, j],
                                         start=(k == 0),
                                         stop=(k == n_acc - 1))
                if extra is not None:
                    ew, ex = extra
                    eblk = ew[:, m * 128:(m + 1) * 128]
                    for j in range(NS):
                        nc.tensor.matmul(ps[:, j], eblk, ex[:, j],
                                         start=False, stop=True)

            def dense(out_pool, win, x_tiles, bias_col, act, out_dt=wdt):
                """[2 x [128,NS,S]] tiles: act(win.T @ x + bias)."""
                outs = []
                for m in range(2):
                    ps = psum_pool.tile([128, NS, S], f32, tag="ps",
                                        name=f"ps{m}")
                    mm_chunk(ps, win, x_tiles, m)
                    o = out_pool.tile([128, NS, S], out_dt, tag="z",
                                      name=f"z{m}")
                    nc.scalar.activation(o[:], ps[:], act,
                                         bias=bcol(m, bias_col))
                    outs.append(o)
                return outs

            groups = [range(g * GROUP, (g + 1) * GROUP)
                      for g in range(N_CHUNKS // GROUP)]

            for gi, chunks in enumerate(groups):
                ep = psum_pool
                # ---- ODE: 5 Euler steps, layer-sweeps within the group ----
                for s in range(NUM_STEPS):
                    z1s = {}
                    z2s = {}
                    for c in chunks:
                        z1s[c] = dense(z1_pool, w1, h[c], COL_B1S + s, Tanh)
                    for c in chunks:
                        z2s[c] = dense(z2_pool, w2, z1s[c], COL_B2, Tanh)
                    for c in chunks:
                        for m in range(2):
                            ps = psum_pool.tile([128, NS, S], f32, tag="ps",
                                                name=f"ps3{m}")
                            mm_chunk(ps, w3, z2s[c], m)
                            # h += (psum + b3)  (rounds h to wdt on store)
                            nc.vector.scalar_tensor_tensor(
                                h[c][m][:], ps[:], bcol(m, COL_B3),
                                h[c][m][:], op0=add, op1=add)

                # ---- event update for this group: three chunk-sweeps so
                # each keeps psum demand at 2 tiles/chunk and pipelines ----
                efs = {}
                gates = {}
                for c in chunks:
                    # ef duplicated into both partition halves (see load_evt_w)
                    efc = ef_pool.tile([128, NS, S], wdt, tag="ef",
                                       name=f"ef{c}")
                    for half in range(2):
                        nc.sync.dma_start(
                            efc[half * EVT:(half + 1) * EVT],
                            efT_d.ap()[:, c * CHUNK:(c + 1) * CHUNK])
                    efs[c] = efc
                    # gate = sigmoid(wg.T @ ef + bg); the two m-halves are
                    # K=64 matmuls placed on distinct PE row groups so they
                    # run concurrently.
                    psg = [ep.tile([128, NS, S], f32, tag="ps",
                                   name=f"psg{m}") for m in range(2)]
                    for j in range(NS):
                        for m in range(2):
                            nc.tensor.matmul(
                                psg[m][:, j],
                                wg[m * EVT:(m + 1) * EVT,
                                   m * 128:(m + 1) * 128],
                                efc[m * EVT:(m + 1) * EVT, j],
                                start=True, stop=True,
                                tile_position=(64 * m, 0))
                    gs = []
                    for m in range(2):
                        gate = z2_pool.tile([128, NS, S], f32, tag="z",
                                            name=f"g{c}_{m}")
                        nc.scalar.activation(gate[:], psg[m][:], Sigmoid,
                                             bias=bcol(m, COL_BG))
                        gs.append(gate)
                    gates[c] = gs
                u1s = {}
                for c in chunks:
                    # u1[m] = relu(we1h.T @ h + we1e.T @ ef + be1) on DVE;
                    # the two m-halves' we1e (K=64) matmuls run concurrently.
                    psu = [ep.tile([128, NS, S], f32, tag="ps",
                                   name=f"psu{m}") for m in range(2)]
                    for m in range(2):
                        for k in range(2):
                            wblk = we1h[k][:, m * 128:(m + 1) * 128]
                            for j in range(NS):
                                nc.tensor.matmul(psu[m][:, j], wblk,
                                                 h[c][k][:, j],
                                                 start=(k == 0), stop=False)
                    for j in range(NS):
                        for m in range(2):
                            nc.tensor.matmul(
                                psu[m][:, j],
                                we1e[m * EVT:(m + 1) * EVT,
                                     m * 128:(m + 1) * 128],
                                efs[c][m * EVT:(m + 1) * EVT, j],
                                start=False, stop=True,
                                tile_position=(64 * m, 0))
                    ts = []
                    for m in range(2):
                        o = z1_pool.tile([128, NS, S], wdt, tag="z",
                                         name=f"u{c}_{m}")
                        # relu on ACT here: during the event DVE is the
                        # busier engine, ACT only runs the sigmoids
                        nc.scalar.activation(o[:], psu[m][:], Relu,
                                             bias=bcol(m, COL_BE1))
                        ts.append(o)
                    u1s[c] = ts
                for c in chunks:
                    for m in range(2):
                        psp = ep.tile([128, NS, S], f32, tag="ps",
                                      name=f"psp{m}")
                        mm_chunk(psp, we2, u1s[c], m)
                        # tmp = (psum_upd + be2) * gate
                        tmp = z2_pool.tile([128, NS, S], f32, tag="z",
                                           name=f"t{c}_{m}")
                        nc.vector.scalar_tensor_tensor(
                            tmp[:], psp[:], bcol(m, COL_BE2),
                            gates[c][m][:], op0=add, op1=mult)
                        # out = tmp + h, plain HWDGE store (an accum-DMA
                        # variant tripled outT traffic and cost ~20us of
                        # SWDGE drain at the kernel tail)
                        stg = stage_pool.tile([128, NS, S], f32, tag="st",
                                              name=f"s{c}_{m}")
                        # split the final adds across DVE and the otherwise
                        # idle GpSimd so the end-of-kernel DVE backlog halves;
                        # GpSimd ops are ~2.4x slower, so keep the last
                        # chunk's adds on DVE so a slow op never ends the run
                        eng = (nc.gpsimd if (m == 1 and c != chunks[-1])
                               else nc.vector)
                        eng.tensor_add(stg[:], tmp[:], h[c][m][:])
                        nc.sync.dma_start(
                            outT_d.ap()[m * 128:(m + 1) * 128,
                                        c * CHUNK:(c + 1) * CHUNK],
                            stg[:])

    nc.finalize()
    return nc


def _get_nc(mode):
    if mode not in _CACHE:
        _CACHE[mode] = _build(mode)
    return _CACHE[mode]


LAST_RESULT = None


def kernel(h_prev, event_features, t_start, t_end,
           Wt1, bt1, Wt2, bt2,
           Wd1, bd1, Wd2, bd2, Wd3, bd3,
           We1, be1, We2, be2, Wg, bg):
    global LAST_RESULT
    assert h_prev.shape == (B, HID) and event_features.shape == (B, EVT)

    # ---- host-side folding (float64 for exactness, cast down once) ----
    f8 = np.float64
    dt = (f8(t_end) - f8(t_start)) / NUM_STEPS
    b1s = np.empty((HID, NUM_STEPS), dtype=f8)
    for s in range(NUM_STEPS):
        t = f8(t_start) + s * dt
        te = np.tanh(t * Wt1[:, 0].astype(f8) + bt1.astype(f8))
        te = Wt2.astype(f8) @ te + bt2.astype(f8)
        b1s[:, s] = bd1.astype(f8) + Wd1[:, HID:].astype(f8) @ te

    xdt = np.float16 if MODE == "f16" else np.float32
    w1T = np.ascontiguousarray(Wd1[:, :HID].T, dtype=xdt)
    w2T = np.ascontiguousarray(Wd2.T, dtype=xdt)
    w3T = np.ascontiguousarray((dt * Wd3.astype(f8)).T.astype(xdt))
    we1hT = np.ascontiguousarray(We1[:, :HID].T, dtype=xdt)
    we1eT = np.ascontiguousarray(We1[:, HID:].T, dtype=xdt)
    we2T = np.ascontiguousarray(We2.T, dtype=xdt)
    wgT = np.ascontiguousarray(Wg.T, dtype=xdt)

    biasp = np.zeros((HID, N_BIAS_COLS), dtype=f8)
    biasp[:, COL_B1S:COL_B1S + NUM_STEPS] = b1s
    biasp[:, COL_B2] = bd2.astype(f8)
    biasp[:, COL_B3] = dt * bd3.astype(f8)
    biasp[:, COL_BE1] = be1.astype(f8)
    biasp[:, COL_BE2] = be2.astype(f8)
    biasp[:, COL_BG] = bg.astype(f8)
    biasp = biasp.astype(np.float32)

    hT = np.ascontiguousarray(h_prev.T, dtype=xdt)      # [HID, B]
    efT = np.ascontiguousarray(event_features.T, dtype=xdt)

    shared = dict(w1=w1T, w2=w2T, w3=w3T, we1h=we1hT, we1e=we1eT,
                  we2=we2T, wg=wgT, biasp=biasp)
    in_maps = []
    for c in range(N_CORES):
        sl = slice(c * R, (c + 1) * R)
        in_maps.append(dict(
            hT=np.ascontiguousarray(hT[:, sl]),
            efT=np.ascontiguousarray(efT[:, sl]),
            **shared))

    nc = _get_nc(MODE)
    # First execution of a freshly-loaded NEFF occasionally faults the
    # exec unit (transient); retry recovers.
    last_err = None
    for _ in range(3):
        try:
            res = run_bass_kernel_spmd(nc, in_maps,
                                       core_ids=list(range(N_CORES)))
            break
        except Exception as e:  # noqa: BLE001
            last_err = e
            # a traced first execution can fault the exec unit; never trace
            # on retries
            os.environ["BASS_NEVER_TRACE"] = "1"
            import time
            time.sleep(2)
    else:
        raise last_err
    LAST_RESULT = res

    out = np.empty((B, HID), dtype=np.float32)
    for c in range(N_CORES):
        out[c * R:(c + 1) * R, :] = res.results[c]["outT"].T
    return out

